# revision 7
# baseline (speedup 1.0000x reference)
"""Channel-attention module kernel for 8 Trainium2 NeuronCores.

reference semantics (B=2, C=128, N=D*H*W=147456):
    q = x.reshape(B, C, N)
    energy = q @ q^T                  # [B, C, C]
    attn = softmax(rowmax(energy) - energy, axis=-1)
          = softmax(-energy, axis=-1)             (rowmax shift is a no-op)
    out = attn @ q
    return x + gamma * out

Sharding: sequence-parallel over N. Core r owns columns
[r*N/8, (r+1)*N/8) of q for both batches. Each core computes a partial
energy (contraction over its local n), an AllReduce sums the tiny
[B, C, C] energy across the 8 cores, each core then computes the
softmax redundantly and applies attn to its local columns.

On-core dataflow per batch:
  - DMA x chunks [128, 2048] to SBUF (kept resident for phase 2)
  - PE-transpose each [128, 128] subtile -> qT in PSUM -> DVE copy to SBUF
  - fp32 matmul accumulate energy += qT^T @ qT into PSUM (precision: the
    softmax argmax gap can be as small as 0.03 on these inputs, so the
    energy contraction must be true fp32)
  - AllReduce energy, softmax with exp(min_row - E) trick, fold
    gamma/Z into the attention weights, PE-transpose attn
  - matmul out = attnT^T @ x_chunk, final = x_chunk + out (one DVE add),
    DMA out
"""

import sys

sys.path.insert(0, "/opt/trn_rl_repo")

import numpy as np

B, C = 2, 128
D, H, W = 16, 96, 96
N = D * H * W  # 147456
NCORES = 8
NLOC = N // NCORES  # 18432
CHUNK = 2048
NCHUNK = NLOC // CHUNK  # 9
OTILE = 512
PIPE = 2  # transposes emitted ahead of their matmul (keeps PE fed)

_compiled = {}


def _log(msg):
    import time as _t
    print(f"[kernel {_t.strftime('%H:%M:%S')}] {msg}", flush=True)


def _build():
    import concourse.bacc as bacc
    import concourse.tile as tile
    import concourse.mybir as mybir

    _log("build start")

    f32 = mybir.dt.float32
    nc = bacc.Bacc("TRN2", target_bir_lowering=False, debug=False,
                   num_devices=NCORES)

    x_d = nc.dram_tensor("x", [B, C, NLOC], f32, kind="ExternalInput").ap()
    g_d = nc.dram_tensor("gamma_col", [C, 1], f32, kind="ExternalInput").ap()
    id_d = nc.dram_tensor("ident", [C, C], f32, kind="ExternalInput").ap()
    o_d = nc.dram_tensor("out", [B, C, NLOC], f32, kind="ExternalOutput").ap()

    with tile.TileContext(nc) as tc:
        with (
            tc.tile_pool(name="xres", bufs=B * NCHUNK) as xp,
            tc.tile_pool(name="qt", bufs=4) as qtp,
            tc.tile_pool(name="tps", bufs=3, space="PSUM") as tps,
            tc.tile_pool(name="eps", bufs=2, space="PSUM") as eps,
            tc.tile_pool(name="ops", bufs=2, space="PSUM") as ops,
            tc.tile_pool(name="misc", bufs=1) as mp,
            tc.tile_pool(name="ost", bufs=3) as ostp,
            tc.tile_pool(name="dram", bufs=2, space="DRAM") as dramp,
        ):
            ident = mp.tile([C, C], f32, name="ident_sb")
            nc.sync.dma_start(ident[:], id_d[:])
            gcol = mp.tile([C, 1], f32, name="gcol")
            nc.sync.dma_start(gcol[:], g_d[:])

            xs = [[xp.tile([C, CHUNK], f32, name=f"x_{b}_{k}", tag="x")
                   for k in range(NCHUNK)] for b in range(B)]
            for b in range(B):
                for k in range(NCHUNK):
                    nc.sync.dma_start(xs[b][k][:],
                                      x_d[b, :, k * CHUNK:(k + 1) * CHUNK])

            energy_cat = mp.tile([C, B * C], f32, name="energy_cat")

            # ---- phase 1: partial energy per batch ----
            ntile = NCHUNK * (CHUNK // C)  # 144 n-tiles of 128 per batch
            for b in range(B):
                e_ps = eps.tile([C, C], f32, name=f"e_ps{b}", tag="e")
                pend = []  # qT tiles transposed but not yet matmul'ed
                mm = 0

                def flush(b=b, e_ps=e_ps):
                    nonlocal mm
                    qt = pend.pop(0)
                    nc.tensor.matmul(e_ps[:], qt[:], qt[:],
                                     start=(mm == 0), stop=(mm == ntile - 1))
                    mm += 1

                for t in range(ntile):
                    k, j = divmod(t, CHUNK // C)
                    tp = tps.tile([C, C], f32, name=f"tp_{b}_{t}", tag="tp")
                    nc.tensor.transpose(
                        tp[:], xs[b][k][:, j * C:(j + 1) * C], ident[:])
                    qt = qtp.tile([C, C], f32, name=f"qt_{b}_{t}", tag="qt")
                    nc.vector.tensor_copy(qt[:], tp[:])
                    pend.append(qt)
                    if len(pend) > PIPE:
                        flush()
                while pend:
                    flush()
                nc.vector.tensor_copy(energy_cat[:, b * C:(b + 1) * C],
                                      e_ps[:])

            # ---- AllReduce the [C, B*C] partial energies ----
            ar_in = dramp.tile([C, B * C], f32, name="ar_in")
            ar_out = dramp.tile([C, B * C], f32, name="ar_out",
                                addr_space="Shared")
            nc.sync.dma_start(ar_in[:], energy_cat[:])
            nc.gpsimd.collective_compute(
                "AllReduce", mybir.AluOpType.add,
                replica_groups=[list(range(NCORES))],
                ins=[ar_in.opt()], outs=[ar_out.opt()],
            )
            energy_red = mp.tile([C, B * C], f32, name="energy_red")
            nc.sync.dma_start(energy_red[:], ar_out[:])

            # ---- phase 2: softmax + apply ----
            for b in range(B):
                E_b = energy_red[:, b * C:(b + 1) * C]
                mcol = mp.tile([C, 1], f32, name=f"mcol{b}")
                nc.vector.tensor_reduce(mcol[:], E_b, axis=mybir.AxisListType.X,
                                        op=mybir.AluOpType.min)
                P_b = mp.tile([C, C], f32, name=f"P{b}")
                zcol = mp.tile([C, 1], f32, name=f"zcol{b}")
                # P = exp(min_row - E), zcol = rowsum(P); exponents <= 0
                nc.scalar.activation(P_b[:], E_b,
                                     mybir.ActivationFunctionType.Exp,
                                     bias=mcol[:], scale=-1.0,
                                     accum_out=zcol[:])
                rz = mp.tile([C, 1], f32, name=f"rz{b}")
                nc.vector.reciprocal(rz[:], zcol[:])
                scol = mp.tile([C, 1], f32, name=f"scol{b}")
                nc.vector.tensor_tensor(scol[:], rz[:], gcol[:],
                                        op=mybir.AluOpType.mult)
                # fold gamma / Z into the weights
                nc.vector.tensor_scalar_mul(P_b[:], P_b[:], scol[:])
                tp2 = tps.tile([C, C], f32, name=f"tpP{b}", tag="tp")
                nc.tensor.transpose(tp2[:], P_b[:], ident[:])
                attnT = mp.tile([C, C], f32, name=f"attnT{b}")
                nc.vector.tensor_copy(attnT[:], tp2[:])

                for k in range(NCHUNK):
                    ost = ostp.tile([C, CHUNK], f32, name=f"ost_{b}_{k}",
                                    tag="ost")
                    for j in range(CHUNK // OTILE):
                        op = ops.tile([C, OTILE], f32, name=f"op_{b}_{k}_{j}",
                                      tag="op")
                        nc.tensor.matmul(
                            op[:], attnT[:],
                            xs[b][k][:, j * OTILE:(j + 1) * OTILE],
                            start=True, stop=True)
                        nc.vector.tensor_tensor(
                            ost[:, j * OTILE:(j + 1) * OTILE], op[:],
                            xs[b][k][:, j * OTILE:(j + 1) * OTILE],
                            op=mybir.AluOpType.add)
                    nc.sync.dma_start(o_d[b, :, k * CHUNK:(k + 1) * CHUNK],
                                      ost[:])

    _log("tile context done; bacc compile start")
    nc.compile()
    _log("bacc compile done")
    return nc


def _get_nc():
    if "nc" not in _compiled:
        _compiled["nc"] = _build()
    return _compiled["nc"]


def kernel(x, gamma, _trace=False, _tmpdir=None):
    from concourse import bass_utils

    x = np.ascontiguousarray(np.asarray(x), dtype=np.float32)
    gamma = np.asarray(gamma, dtype=np.float32)
    q = x.reshape(B, C, N)
    gcol = np.full((C, 1), gamma[0], dtype=np.float32)
    ident = np.eye(C, dtype=np.float32)

    in_maps = []
    for r in range(NCORES):
        in_maps.append({
            "x": np.ascontiguousarray(q[:, :, r * NLOC:(r + 1) * NLOC]),
            "gamma_col": gcol,
            "ident": ident,
        })

    nc = _get_nc()
    _log("launching run_bass_kernel_spmd")
    res = bass_utils.run_bass_kernel_spmd(
        nc, in_maps, core_ids=list(range(NCORES)), trace=_trace,
        tmpdir=_tmpdir)
    outs = [res.results[r]["out"] for r in range(NCORES)]
    full = np.concatenate(outs, axis=2).reshape(B, C, D, H, W)
    if _trace:
        return full.astype(np.float32, copy=False), res
    return full.astype(np.float32, copy=False)


# revision 9
# speedup vs baseline: 1.1523x; 1.1523x over previous
"""Channel-attention module kernel for 8 Trainium2 NeuronCores.

reference semantics (B=2, C=128, N=D*H*W=147456):
    q = x.reshape(B, C, N)
    energy = q @ q^T                  # [B, C, C]
    attn = softmax(rowmax(energy) - energy, axis=-1)
          = softmax(-energy, axis=-1)             (rowmax shift is a no-op)
    out = attn @ q
    return x + gamma * out

Sharding: sequence-parallel over N. Core r owns columns
[r*N/8, (r+1)*N/8) of q for both batches. Each core computes a partial
energy (contraction over its local n), a per-batch AllReduce sums the
tiny [C, C] energy across the 8 cores, each core then computes the
softmax redundantly and applies the attention to its local columns.

Pipelining: energy(b0) -> AR(b0) overlaps energy(b1); AR(b1) overlaps
phase2(b0). The residual is folded into the attention matrix
(attn_s = gamma/Z * P + I, where P's diagonal is exactly 0 because the
energy diagonal ~ +N dominates), so phase 2 per output tile is one
matmul plus one PSUM->SBUF copy (alternating DVE / ScalarE) and a DMA.

Precision: the energy contraction must be true fp32 (softmax argmin
gaps as small as 0.03 on these inputs; an argmin flip alone costs ~5%
global rel err). Phase-2 matmuls run as float32r (reduced-precision
fp32 at 4x the fp32 matmul rate for free dim >= 256) — error there is
linear, ~1e-3, far inside the 2e-2 gate.
"""

import sys

sys.path.insert(0, "/opt/trn_rl_repo")

import numpy as np

B, C = 2, 128
D, H, W = 16, 96, 96
N = D * H * W  # 147456
NCORES = 8
NLOC = N // NCORES  # 18432
CHUNK = 2048
NCHUNK = NLOC // CHUNK  # 9
OTILE = 512
PIPE = 2  # transposes emitted ahead of their matmul (keeps PE fed)
FP32R_PHASE2 = False  # bitcast-only f32r rejected by BIR verifier; needs cast-copies

_compiled = {}


def _log(msg):
    import time as _t
    print(f"[kernel {_t.strftime('%H:%M:%S')}] {msg}", flush=True)


def _build():
    import concourse.bacc as bacc
    import concourse.tile as tile
    import concourse.mybir as mybir

    _log("build start")

    f32 = mybir.dt.float32
    f32r = mybir.dt.float32r
    nc = bacc.Bacc("TRN2", target_bir_lowering=False, debug=False,
                   num_devices=NCORES)

    x_d = nc.dram_tensor("x", [B, C, NLOC], f32, kind="ExternalInput").ap()
    g_d = nc.dram_tensor("gamma_col", [C, 1], f32, kind="ExternalInput").ap()
    id_d = nc.dram_tensor("ident", [C, C], f32, kind="ExternalInput").ap()
    o_d = nc.dram_tensor("out", [B, C, NLOC], f32, kind="ExternalOutput").ap()

    with tile.TileContext(nc) as tc:
        with (
            tc.tile_pool(name="xres", bufs=B * NCHUNK) as xp,
            tc.tile_pool(name="qt", bufs=4) as qtp,
            tc.tile_pool(name="tps", bufs=3, space="PSUM") as tps,
            tc.tile_pool(name="eps", bufs=2, space="PSUM") as eps,
            tc.tile_pool(name="ops", bufs=3, space="PSUM") as ops,
            tc.tile_pool(name="misc", bufs=1) as mp,
            tc.tile_pool(name="ost", bufs=3) as ostp,
            tc.tile_pool(name="dram", bufs=1, space="DRAM") as dramp,
        ):
            ident = mp.tile([C, C], f32, name="ident_sb")
            nc.sync.dma_start(ident[:], id_d[:])
            gcol = mp.tile([C, 1], f32, name="gcol")
            nc.sync.dma_start(gcol[:], g_d[:])

            xs = [[xp.tile([C, CHUNK], f32, name=f"x_{b}_{k}", tag="x")
                   for k in range(NCHUNK)] for b in range(B)]
            for b in range(B):
                for k in range(NCHUNK):
                    nc.sync.dma_start(xs[b][k][:],
                                      x_d[b, :, k * CHUNK:(k + 1) * CHUNK])

            # ---- phase 1 + per-batch AllReduce ----
            ntile = NCHUNK * (CHUNK // C)  # 144 n-tiles of 128 per batch
            E_sb = []
            for b in range(B):
                e_ps = eps.tile([C, C], f32, name=f"e_ps{b}", tag="e")
                pend = []
                mm = 0

                def flush(e_ps=e_ps):
                    nonlocal mm
                    qt = pend.pop(0)
                    nc.tensor.matmul(e_ps[:], qt[:], qt[:],
                                     start=(mm == 0), stop=(mm == ntile - 1))
                    mm += 1

                for t in range(ntile):
                    k, j = divmod(t, CHUNK // C)
                    tp = tps.tile([C, C], f32, name=f"tp_{b}_{t}", tag="tp")
                    nc.tensor.transpose(
                        tp[:], xs[b][k][:, j * C:(j + 1) * C], ident[:])
                    qt = qtp.tile([C, C], f32, name=f"qt_{b}_{t}", tag="qt")
                    nc.vector.tensor_copy(qt[:], tp[:])
                    pend.append(qt)
                    if len(pend) > PIPE:
                        flush()
                while pend:
                    flush()
                e_cat = mp.tile([C, C], f32, name=f"e_cat{b}")
                nc.vector.tensor_copy(e_cat[:], e_ps[:])

                ar_in = dramp.tile([C, C], f32, name=f"ar_in{b}")
                ar_out = dramp.tile([C, C], f32, name=f"ar_out{b}",
                                    addr_space="Shared")
                nc.sync.dma_start(ar_in[:], e_cat[:])
                nc.gpsimd.collective_compute(
                    "AllReduce", mybir.AluOpType.add,
                    replica_groups=[list(range(NCORES))],
                    ins=[ar_in.opt()], outs=[ar_out.opt()],
                )
                e_red = mp.tile([C, C], f32, name=f"e_red{b}")
                nc.sync.dma_start(e_red[:], ar_out[:])
                E_sb.append(e_red)

            # ---- phase 2: softmax + apply, per batch ----
            for b in range(B):
                E_b = E_sb[b][:]
                mcol = mp.tile([C, 1], f32, name=f"mcol{b}")
                nc.vector.tensor_reduce(mcol[:], E_b, axis=mybir.AxisListType.X,
                                        op=mybir.AluOpType.min)
                P_b = mp.tile([C, C], f32, name=f"P{b}")
                zcol = mp.tile([C, 1], f32, name=f"zcol{b}")
                # P = exp(min_row - E), zcol = rowsum(P); exponents <= 0.
                # P's diagonal is exp(min - ~+147000) == 0 exactly.
                nc.scalar.activation(P_b[:], E_b,
                                     mybir.ActivationFunctionType.Exp,
                                     bias=mcol[:], scale=-1.0,
                                     accum_out=zcol[:])
                rz = mp.tile([C, 1], f32, name=f"rz{b}")
                nc.vector.reciprocal(rz[:], zcol[:])
                scol = mp.tile([C, 1], f32, name=f"scol{b}")
                nc.vector.tensor_tensor(scol[:], rz[:], gcol[:],
                                        op=mybir.AluOpType.mult)
                # attn_s = (gamma/Z) * P + I  -> matmul computes x + gamma*attn@q
                nc.vector.tensor_scalar_mul(P_b[:], P_b[:], scol[:])
                nc.vector.tensor_add(P_b[:], P_b[:], ident[:])
                tp2 = tps.tile([C, C], f32, name=f"tpP{b}", tag="tp")
                nc.tensor.transpose(tp2[:], P_b[:], ident[:])
                attnT = mp.tile([C, C], f32, name=f"attnT{b}")
                nc.vector.tensor_copy(attnT[:], tp2[:])
                if FP32R_PHASE2:
                    attnT_mm = attnT.bitcast(f32r)
                else:
                    attnT_mm = attnT

                for k in range(NCHUNK):
                    ost = ostp.tile([C, CHUNK], f32, name=f"ost_{b}_{k}",
                                    tag="ost")
                    for j in range(CHUNK // OTILE):
                        op = ops.tile([C, OTILE], f32, name=f"op_{b}_{k}_{j}",
                                      tag="op")
                        rhs = xs[b][k][:, j * OTILE:(j + 1) * OTILE]
                        if FP32R_PHASE2:
                            rhs = rhs.bitcast(f32r)
                        nc.tensor.matmul(op[:], attnT_mm[:], rhs,
                                         start=True, stop=True)
                        dst = ost[:, j * OTILE:(j + 1) * OTILE]
                        if j % 2 == 0:
                            nc.vector.tensor_copy(dst, op[:])
                        else:
                            nc.scalar.copy(dst, op[:])
                    nc.sync.dma_start(o_d[b, :, k * CHUNK:(k + 1) * CHUNK],
                                      ost[:])

    _log("tile context done; bacc compile start")
    nc.compile()
    _log("bacc compile done")
    return nc


def _get_nc():
    if "nc" not in _compiled:
        _compiled["nc"] = _build()
    return _compiled["nc"]


def kernel(x, gamma, _trace=False, _tmpdir=None):
    from concourse import bass_utils

    x = np.ascontiguousarray(np.asarray(x), dtype=np.float32)
    gamma = np.asarray(gamma, dtype=np.float32)
    q = x.reshape(B, C, N)
    gcol = np.full((C, 1), gamma[0], dtype=np.float32)
    ident = np.eye(C, dtype=np.float32)

    in_maps = []
    for r in range(NCORES):
        in_maps.append({
            "x": np.ascontiguousarray(q[:, :, r * NLOC:(r + 1) * NLOC]),
            "gamma_col": gcol,
            "ident": ident,
        })

    nc = _get_nc()
    _log("launching run_bass_kernel_spmd")
    res = bass_utils.run_bass_kernel_spmd(
        nc, in_maps, core_ids=list(range(NCORES)), trace=_trace,
        tmpdir=_tmpdir)
    outs = [res.results[r]["out"] for r in range(NCORES)]
    full = np.concatenate(outs, axis=2).reshape(B, C, D, H, W)
    if _trace:
        return full.astype(np.float32, copy=False), res
    return full.astype(np.float32, copy=False)


# revision 10
# speedup vs baseline: 1.2436x; 1.0792x over previous
"""Channel-attention module kernel for 8 Trainium2 NeuronCores.

reference semantics (B=2, C=128, N=D*H*W=147456):
    q = x.reshape(B, C, N)
    energy = q @ q^T                  # [B, C, C]
    attn = softmax(rowmax(energy) - energy, axis=-1)
          = softmax(-energy, axis=-1)             (rowmax shift is a no-op)
    out = attn @ q
    return x + gamma * out

Sharding: sequence-parallel over N. Core r owns columns
[r*N/8, (r+1)*N/8) of q for both batches. Each core computes a partial
energy (contraction over its local n), a per-batch AllReduce sums the
tiny [C, C] energy across the 8 cores, each core then computes the
softmax redundantly and applies the attention to its local columns.

Pipelining: energy(b0) -> AR(b0) overlaps energy(b1); AR(b1) overlaps
phase2(b0).

Precision split:
  - energy contraction: true fp32 (softmax argmin gaps as small as 0.03
    on these inputs; one argmin flip alone is ~5% global rel err).
  - phase 2 (attn apply): bf16. The residual is folded into the
    attention matrix (attn_s = gamma/Z * P + I; P's diagonal is exactly
    0 because the energy diagonal ~ +N dominates), so phase 2 is
    out = attn_s @ q with q rounded to bf16 — error is linear, ~0.4%,
    far inside the 2e-2 gate. This makes phase-2 matmuls 4x faster than
    fp32 and lets the fp32 x chunks be freed after phase 1: x lives in
    a small fp32 ring; a resident bf16 copy (cast on the idle ScalarE
    during phase 1) feeds phase 2.
"""

import sys

sys.path.insert(0, "/opt/trn_rl_repo")

import numpy as np

B, C = 2, 128
D, H, W = 16, 96, 96
N = D * H * W  # 147456
NCORES = 8
NLOC = N // NCORES  # 18432
CHUNK = 2048
NCHUNK = NLOC // CHUNK  # 9
OTILE = 512
PIPE = 2  # transposes emitted ahead of their matmul (keeps PE fed)

_compiled = {}


def _log(msg):
    import time as _t
    print(f"[kernel {_t.strftime('%H:%M:%S')}] {msg}", flush=True)


def _build():
    import concourse.bacc as bacc
    import concourse.tile as tile
    import concourse.mybir as mybir

    _log("build start")

    f32 = mybir.dt.float32
    bf16 = mybir.dt.bfloat16
    nc = bacc.Bacc("TRN2", target_bir_lowering=False, debug=False,
                   num_devices=NCORES)

    x_d = nc.dram_tensor("x", [B, C, NLOC], f32, kind="ExternalInput").ap()
    g_d = nc.dram_tensor("gamma_col", [C, 1], f32, kind="ExternalInput").ap()
    id_d = nc.dram_tensor("ident", [C, C], f32, kind="ExternalInput").ap()
    o_d = nc.dram_tensor("out", [B, C, NLOC], f32, kind="ExternalOutput").ap()

    with tile.TileContext(nc) as tc:
        with (
            tc.tile_pool(name="xring", bufs=5) as xp,
            tc.tile_pool(name="xb16", bufs=B * NCHUNK) as xbp,
            tc.tile_pool(name="qt", bufs=4) as qtp,
            tc.tile_pool(name="tps", bufs=3, space="PSUM") as tps,
            tc.tile_pool(name="eps", bufs=2, space="PSUM") as eps,
            tc.tile_pool(name="ops", bufs=3, space="PSUM") as ops,
            tc.tile_pool(name="misc", bufs=1) as mp,
            tc.tile_pool(name="ost", bufs=3) as ostp,
            tc.tile_pool(name="dram", bufs=1, space="DRAM") as dramp,
        ):
            ident = mp.tile([C, C], f32, name="ident_sb")
            nc.sync.dma_start(ident[:], id_d[:])
            gcol = mp.tile([C, 1], f32, name="gcol")
            nc.sync.dma_start(gcol[:], g_d[:])

            xb16 = [[xbp.tile([C, CHUNK], bf16, name=f"xb_{b}_{k}", tag="xb")
                     for k in range(NCHUNK)] for b in range(B)]

            # ---- phase 1 + per-batch AllReduce ----
            ntile_c = CHUNK // C  # 16 n-tiles of 128 per chunk
            ntile = NCHUNK * ntile_c  # 144 per batch
            E_sb = []
            for b in range(B):
                e_ps = eps.tile([C, C], f32, name=f"e_ps{b}", tag="e")
                pend = []
                mm = 0

                def flush(e_ps=e_ps):
                    nonlocal mm
                    qt = pend.pop(0)
                    nc.tensor.matmul(e_ps[:], qt[:], qt[:],
                                     start=(mm == 0), stop=(mm == ntile - 1))
                    mm += 1

                for k in range(NCHUNK):
                    xt = xp.tile([C, CHUNK], f32, name=f"x_{b}_{k}", tag="x")
                    src = x_d[b, :, k * CHUNK:(k + 1) * CHUNK]
                    if b == 0 and k == 0:
                        # split the very first load so PE can start early
                        for s in range(4):
                            nc.sync.dma_start(
                                xt[:, s * 512:(s + 1) * 512],
                                x_d[0, :, s * 512:(s + 1) * 512])
                    else:
                        nc.sync.dma_start(xt[:], src)
                    for j in range(ntile_c):
                        t = k * ntile_c + j
                        tp = tps.tile([C, C], f32, name=f"tp_{b}_{t}",
                                      tag="tp")
                        nc.tensor.transpose(
                            tp[:], xt[:, j * C:(j + 1) * C], ident[:])
                        qt = qtp.tile([C, C], f32, name=f"qt_{b}_{t}",
                                      tag="qt")
                        nc.vector.tensor_copy(qt[:], tp[:])
                        pend.append(qt)
                        if len(pend) > PIPE:
                            flush()
                    # bf16 copy for phase 2 (ScalarE is idle in phase 1);
                    # after this the fp32 ring slot can be reused.
                    nc.scalar.copy(xb16[b][k][:], xt[:])
                while pend:
                    flush()
                e_cat = mp.tile([C, C], f32, name=f"e_cat{b}")
                nc.vector.tensor_copy(e_cat[:], e_ps[:])

                ar_in = dramp.tile([C, C], f32, name=f"ar_in{b}")
                ar_out = dramp.tile([C, C], f32, name=f"ar_out{b}",
                                    addr_space="Shared")
                nc.sync.dma_start(ar_in[:], e_cat[:])
                nc.gpsimd.collective_compute(
                    "AllReduce", mybir.AluOpType.add,
                    replica_groups=[list(range(NCORES))],
                    ins=[ar_in.opt()], outs=[ar_out.opt()],
                )
                e_red = mp.tile([C, C], f32, name=f"e_red{b}")
                nc.sync.dma_start(e_red[:], ar_out[:])
                E_sb.append(e_red)

            # ---- phase 2: softmax + apply, per batch ----
            for b in range(B):
                E_b = E_sb[b][:]
                mcol = mp.tile([C, 1], f32, name=f"mcol{b}")
                nc.vector.tensor_reduce(mcol[:], E_b, axis=mybir.AxisListType.X,
                                        op=mybir.AluOpType.min)
                P_b = mp.tile([C, C], f32, name=f"P{b}")
                zcol = mp.tile([C, 1], f32, name=f"zcol{b}")
                # P = exp(min_row - E), zcol = rowsum(P); exponents <= 0.
                # P's diagonal is exp(min - ~+147000) == 0 exactly.
                nc.scalar.activation(P_b[:], E_b,
                                     mybir.ActivationFunctionType.Exp,
                                     bias=mcol[:], scale=-1.0,
                                     accum_out=zcol[:])
                rz = mp.tile([C, 1], f32, name=f"rz{b}")
                nc.vector.reciprocal(rz[:], zcol[:])
                scol = mp.tile([C, 1], f32, name=f"scol{b}")
                nc.vector.tensor_tensor(scol[:], rz[:], gcol[:],
                                        op=mybir.AluOpType.mult)
                # attn_s = (gamma/Z) * P + I  -> matmul computes x + gamma*attn@q
                nc.vector.tensor_scalar_mul(P_b[:], P_b[:], scol[:])
                nc.vector.tensor_add(P_b[:], P_b[:], ident[:])
                tp2 = tps.tile([C, C], f32, name=f"tpP{b}", tag="tp")
                nc.tensor.transpose(tp2[:], P_b[:], ident[:])
                attnT = mp.tile([C, C], bf16, name=f"attnT{b}")
                nc.vector.tensor_copy(attnT[:], tp2[:])  # fp32 psum -> bf16

                for k in range(NCHUNK):
                    ost = ostp.tile([C, CHUNK], f32, name=f"ost_{b}_{k}",
                                    tag="ost")
                    for j in range(CHUNK // OTILE):
                        op = ops.tile([C, OTILE], f32, name=f"op_{b}_{k}_{j}",
                                      tag="op")
                        nc.tensor.matmul(
                            op[:], attnT[:],
                            xb16[b][k][:, j * OTILE:(j + 1) * OTILE],
                            start=True, stop=True)
                        nc.vector.tensor_copy(
                            ost[:, j * OTILE:(j + 1) * OTILE], op[:])
                    nc.sync.dma_start(o_d[b, :, k * CHUNK:(k + 1) * CHUNK],
                                      ost[:])

    _log("tile context done; bacc compile start")
    nc.compile()
    _log("bacc compile done")
    return nc


def _get_nc():
    if "nc" not in _compiled:
        _compiled["nc"] = _build()
    return _compiled["nc"]


def kernel(x, gamma, _trace=False, _tmpdir=None):
    from concourse import bass_utils

    x = np.ascontiguousarray(np.asarray(x), dtype=np.float32)
    gamma = np.asarray(gamma, dtype=np.float32)
    q = x.reshape(B, C, N)
    gcol = np.full((C, 1), gamma[0], dtype=np.float32)
    ident = np.eye(C, dtype=np.float32)

    in_maps = []
    for r in range(NCORES):
        in_maps.append({
            "x": np.ascontiguousarray(q[:, :, r * NLOC:(r + 1) * NLOC]),
            "gamma_col": gcol,
            "ident": ident,
        })

    nc = _get_nc()
    _log("launching run_bass_kernel_spmd")
    res = bass_utils.run_bass_kernel_spmd(
        nc, in_maps, core_ids=list(range(NCORES)), trace=_trace,
        tmpdir=_tmpdir)
    outs = [res.results[r]["out"] for r in range(NCORES)]
    full = np.concatenate(outs, axis=2).reshape(B, C, D, H, W)
    if _trace:
        return full.astype(np.float32, copy=False), res
    return full.astype(np.float32, copy=False)
